# revision 9
# baseline (speedup 1.0000x reference)
"""Bass/Tile Trainium2 kernel for DotProductAttention with valid_lens masking.

Contract: kernel(**inputs) takes FULL inputs (B=64), shards batch dim over 8
NeuronCores (8 batches/core), computes out = softmax(QK^T/sqrt(d) masked) @ V
and the full attention matrix on-device, gathers FULL outputs.

Returns (out, attn) matching the reference's return structure.

Per-batch pipeline on each core:
  - load Q,K,V natural; PE-transpose Q,K -> [d, seq] (fp32), round to fp32r
    via the ScalarE PSUM->SBUF evacuation copy.
  - S = Qt^T Kt via fp32r matmuls (1 cyc/row), plus a rank-1 ones x maskrow
    matmul accumulated into the same PSUM bank: masked columns get -2^20.
  - ScalarE exp(scale*S) straight from PSUM with fused per-row accumulation
    (the softmax denominator); masked columns underflow to exactly 0.
  - GPSIMD normalizes the row block (per-partition reciprocal multiply) and
    DMA writes it to the attn output.
  - PE transposes the unnormalized block (fp32), DVE rounds it to fp32r on
    evacuation; AV = sum_k V^T A^T accumulates out^T in PSUM (fp32r matmuls);
    final PE transpose + per-partition reciprocal scale gives out rows.
"""

import sys

sys.path.insert(0, "/opt/trn_rl_repo")

import numpy as np

import concourse.bacc as bacc
import concourse.mybir as mybir
import concourse.tile as tile
from concourse.bass_utils import run_bass_kernel_spmd
from concourse.masks import make_identity

B, Q, K, D = 64, 2048, 2048, 128
N_CORES = 8
BL = B // N_CORES  # batches per core
P = 128
QT = Q // P  # 16 q-tiles
KT = K // P  # 16 k-tiles
GW = 2  # q-tiles per AV group (rhs N = GW*128 = 256 >= 256 keeps fp32r at 1 cyc/row)
NG = QT // GW
SCALE = float(np.float32(1.0) / np.sqrt(np.float32(D)))
MASKVAL = -float(2.0**20)  # exactly representable; scale*MASKVAL -> exp == 0.0

F32 = mybir.dt.float32
F32R = mybir.dt.float32r
EXP = mybir.ActivationFunctionType.Exp
ADD = mybir.AluOpType.add
MULT = mybir.AluOpType.mult




def build_program(bl=BL, au_f32r=True, ps_s_bufs=2, at_chunk=512, norm_dve_every=4, evac_act_every=0):
    AT_J = at_chunk // P  # transposes per staging tile
    AU_DT = F32R if au_f32r else F32
    nc = bacc.Bacc()
    q_d = nc.dram_tensor("queries", [bl, Q, D], F32R, kind="ExternalInput")
    k_d = nc.dram_tensor("keys", [bl, K, D], F32R, kind="ExternalInput")
    v_d = nc.dram_tensor("values", [bl, K, D], F32R, kind="ExternalInput")
    m_d = nc.dram_tensor("maskbias", [bl, K], F32R, kind="ExternalInput")
    out_d = nc.dram_tensor("out", [bl, Q, D], F32, kind="ExternalOutput")
    attn_d = nc.dram_tensor("attn", [bl, Q, K], F32, kind="ExternalOutput")

    with tile.TileContext(nc) as tc:
        with (
            tc.tile_pool(name="singles", bufs=1) as singles,
            tc.tile_pool(name="ld", bufs=1) as ld,
            tc.tile_pool(name="maskp", bufs=2) as maskp,
            tc.tile_pool(name="tr", bufs=2) as tr,
            tc.tile_pool(name="work", bufs=2) as work,
            tc.tile_pool(name="anp", bufs=2) as anp,
            tc.tile_pool(name="atp", bufs=2) as atp,
            tc.tile_pool(name="small", bufs=4) as small,
            tc.tile_pool(name="brecp", bufs=2) as brecp,
            tc.tile_pool(name="outp", bufs=3) as outp,
            tc.tile_pool(name="ps_s", bufs=ps_s_bufs, space="PSUM") as ps_s,
            tc.tile_pool(name="ps_at", bufs=2, space="PSUM") as ps_at,
            tc.tile_pool(name="ps_o", bufs=2, space="PSUM") as ps_o,
        ):
            ident_f32 = singles.tile([P, P], F32)
            make_identity(nc, ident_f32)
            ident = singles.tile([P, P], F32R)
            nc.vector.tensor_copy(ident, ident_f32)
            ones_f32 = singles.tile([1, P], F32)
            nc.gpsimd.memset(ones_f32, 1.0)
            ones1 = singles.tile([1, P], F32R)
            nc.vector.tensor_copy(ones1, ones_f32)

            for b in range(bl):
                # ---- load inputs (natural layout, tiled by 128 rows) ----
                q_nat = ld.tile([P, QT, P], F32R, tag="q_nat")
                k_nat = ld.tile([P, KT, P], F32R, tag="k_nat")
                v_nat = ld.tile([P, KT, P], F32R, tag="v_nat")
                nc.sync.dma_start(
                    out=q_nat, in_=q_d[b].rearrange("(t p) d -> p t d", p=P)
                )
                nc.sync.dma_start(
                    out=k_nat, in_=k_d[b].rearrange("(t p) d -> p t d", p=P)
                )
                nc.sync.dma_start(
                    out=v_nat, in_=v_d[b].rearrange("(t p) d -> p t d", p=P)
                )
                maskrow = maskp.tile([1, K], F32R, tag="maskrow")
                nc.sync.dma_start(out=maskrow, in_=m_d[b : b + 1, :])

                # ---- build Q^T, K^T [d, seq]; ScalarE evac rounds to fp32r ----
                qt_sb = tr.tile([P, Q], F32R, tag="qt_sb")
                kt_sb = tr.tile([P, K], F32R, tag="kt_sb")
                for (src, dst, nt) in ((q_nat, qt_sb, QT), (k_nat, kt_sb, KT)):
                    for h in range(nt * P // at_chunk):
                        pt = ps_at.tile([P, at_chunk], F32R, tag="ps_stage")
                        for j in range(AT_J):
                            t = h * AT_J + j
                            nc.tensor.transpose(
                                pt[:, j * P : (j + 1) * P], src[:, t, :], ident
                            )
                        nc.scalar.copy(dst[:, h * at_chunk : (h + 1) * at_chunk], pt)

                brec = brecp.tile([P, QT], F32, tag="brec")

                for g in range(NG):
                    atg = atp.tile([P, KT, GW * P], F32R, tag="atg")
                    for j_q in range(GW):
                        qt = g * GW + j_q
                        # ---- scores + mask for q-tile, in two [128,1024] halves
                        au = work.tile([P, K], AU_DT, tag="au")
                        sums = small.tile([P, 2], F32, tag="sums")
                        for h in range(2):
                            ps = ps_s.tile([P, 1024], F32, tag="ps_s")
                            for c in range(2):
                                ks = h * 1024 + c * 512
                                nc.tensor.matmul(
                                    ps[:, c * 512 : (c + 1) * 512],
                                    lhsT=qt_sb[:, qt * P : (qt + 1) * P],
                                    rhs=kt_sb[:, ks : ks + 512],
                                    start=True,
                                    stop=False,
                                )
                                nc.tensor.matmul(
                                    ps[:, c * 512 : (c + 1) * 512],
                                    lhsT=ones1,
                                    rhs=maskrow[0:1, ks : ks + 512],
                                    start=False,
                                    stop=True,
                                )
                            nc.scalar.activation(
                                out=au[:, h * 1024 : (h + 1) * 1024],
                                in_=ps,
                                func=EXP,
                                bias=0.0,
                                scale=SCALE,
                                accum_out=sums[:, h : h + 1],
                            )
                        ssum = small.tile([P, 1], F32, tag="ssum")
                        nc.vector.tensor_scalar(
                            out=ssum, in0=sums[:, 0:1], scalar1=sums[:, 1:2],
                            scalar2=None, op0=ADD,
                        )
                        nc.vector.reciprocal(out=brec[:, qt : qt + 1], in_=ssum)

                        # ---- normalized attention row-block -> DRAM ----
                        an = anp.tile([P, K], F32, tag="an")
                        norm_eng = (
                            nc.vector
                            if (norm_dve_every and qt % norm_dve_every == 0)
                            else nc.gpsimd
                        )
                        norm_eng.tensor_scalar(
                            out=an, in0=au, scalar1=brec[:, qt : qt + 1],
                            scalar2=None, op0=MULT,
                        )
                        nc.sync.dma_start(
                            out=attn_d[b, qt * P : (qt + 1) * P, :], in_=an
                        )

                        # ---- transpose unnormalized block for AV (fp32 mode);
                        # DVE evac copy rounds to fp32r for the AV matmul ----
                        for hh in range(KT * P // at_chunk):
                            pat = ps_at.tile([P, at_chunk], AU_DT, tag="ps_stage")
                            for j in range(AT_J):
                                kt = hh * AT_J + j
                                nc.tensor.transpose(
                                    pat[:, j * P : (j + 1) * P],
                                    au[:, kt * P : (kt + 1) * P],
                                    ident if au_f32r else ident_f32,
                                )
                            dst_ap = atg[
                                :, hh * AT_J : (hh + 1) * AT_J, j_q * P : (j_q + 1) * P
                            ]
                            src_ap = pat.rearrange("p (j x) -> p j x", j=AT_J)
                            if evac_act_every and hh % evac_act_every == 0:
                                nc.scalar.copy(dst_ap, src_ap)
                            else:
                                nc.vector.tensor_copy(dst_ap, src_ap)

                    # ---- AV for this group: outT[d, GW*128] = sum_k V^T A^T
                    po = ps_o.tile([P, GW * P], F32, tag="ps_o")
                    for kt in range(KT):
                        nc.tensor.matmul(
                            po,
                            lhsT=v_nat[:, kt, :],
                            rhs=atg[:, kt, :],
                            start=(kt == 0),
                            stop=(kt == KT - 1),
                        )
                    ot_sb = outp.tile([P, GW * P], F32R, tag="ot_sb")
                    nc.vector.tensor_copy(ot_sb, po)
                    pf = ps_o.tile([P, GW * P], F32R, tag="ps_o")
                    for j in range(GW):
                        nc.tensor.transpose(
                            pf[:, j * P : (j + 1) * P],
                            ot_sb[:, j * P : (j + 1) * P],
                            ident,
                        )
                    of_sb = outp.tile([P, GW, P], F32, tag="of_sb")
                    for j in range(GW):
                        qt = g * GW + j
                        nc.vector.tensor_scalar(
                            out=of_sb[:, j, :],
                            in0=pf[:, j * P : (j + 1) * P],
                            scalar1=brec[:, qt : qt + 1],
                            scalar2=None,
                            op0=MULT,
                        )
                    nc.sync.dma_start(
                        out=out_d[b, g * GW * P : (g + 1) * GW * P, :].rearrange(
                            "(j p) d -> p j d", p=P
                        ),
                        in_=of_sb,
                    )

    nc.finalize()
    return nc


_PROGRAM_CACHE = {}


def _get_program(bl=BL):
    if bl not in _PROGRAM_CACHE:
        _PROGRAM_CACHE[bl] = build_program(bl)
    return _PROGRAM_CACHE[bl]


def kernel(queries, keys, values, valid_lens, _trace=False):
    queries = np.ascontiguousarray(np.asarray(queries, dtype=np.float32))
    keys = np.ascontiguousarray(np.asarray(keys, dtype=np.float32))
    values = np.ascontiguousarray(np.asarray(values, dtype=np.float32))
    valid_lens = np.asarray(valid_lens)

    maskbias = np.where(
        np.arange(K, dtype=np.int64)[None, :] < valid_lens.astype(np.int64)[:, None],
        np.float32(0.0),
        np.float32(MASKVAL),
    ).astype(np.float32)

    nc = _get_program(BL)
    in_maps = []
    for c in range(N_CORES):
        sl = slice(c * BL, (c + 1) * BL)
        in_maps.append(
            {
                "queries": queries[sl],
                "keys": keys[sl],
                "values": values[sl],
                "maskbias": maskbias[sl],
            }
        )
    res = run_bass_kernel_spmd(nc, in_maps, list(range(N_CORES)), trace=_trace)
    out = np.concatenate([res.results[c]["out"] for c in range(N_CORES)], axis=0)
    attn = np.concatenate([res.results[c]["attn"] for c in range(N_CORES)], axis=0)

    # valid_len == 0 -> reference yields uniform attention (all scores equal
    # NEG); on-device path divides by a zero sum there, so fix up on host.
    zmask = np.asarray(valid_lens) == 0
    if zmask.any():
        attn[zmask] = np.float32(1.0 / K)
        out[zmask] = np.broadcast_to(
            values[zmask].mean(axis=1, keepdims=True).astype(np.float32),
            (int(zmask.sum()), Q, D),
        )

    if _trace:
        return (out, attn), res
    return (out, attn)
